# revision 2
# baseline (speedup 1.0000x reference)
"""Trainium2 Bass kernel for nn_CAttentionLegacy (channel attention), v2.

Per-batch-element pipeline (1 batch element per NeuronCore, 8 cores):
  Pass 1: fused 3x3 conv (host-precomputed W_f = W_dw compose W_qkv) producing
     ONLY q,k as [spatial, channel] rows (x-patch stationary), K-packed to 14
     matmuls per row via host-pre-shifted duplicate lo-channel planes.
     Gram G = q^T k and sum-of-squares accumulate in PSUM across all rows.
  Middle: norms + per-head softmax -> A; C^T = A^T @ Wp^T; then fold the
     attention into the v-conv weights on-device: Wy^T chunks = Wfv^T @ C^T.
  Pass 2: y = conv3x3(x, Wy) directly (Wy stationary, x moving, N=512),
     K-packed to 28 matmuls per 4-row group.  No v materialization at all.
All matmul operands bf16 (fp32 PSUM accumulation).
"""
import sys
sys.path.insert(0, '/opt/trn_rl_repo')

import time
import numpy as np
import jax

import concourse.bass as bass
import concourse.tile as tile
from concourse import mybir, bass2jax

HEADS = 6
DIM = 192
B = 8
HW = 128 * 128
PW = 130  # padded row length
PLANE = PW * PW
F32 = mybir.dt.float32
BF16 = mybir.dt.bfloat16
AX = mybir.AxisListType
AF = mybir.ActivationFunctionType

# chunk table: 14 K-packed contraction chunks of the 3x3 conv over 192 ci.
# (plane, row_off, col_off): plane 0 = hi (ci 0:128), 1 = l2 (lo | lo<<1col),
# 2 = l3 (lo | lo<<1row).
CHUNKS = [(0, dy, dx) for dy in range(3) for dx in range(3)] + [
    (1, 0, 0), (1, 1, 0), (1, 2, 0),   # pairs {(dy,0),(dy,1)}
    (2, 0, 2),                         # pair {(0,2),(1,2)}
    (1, 2, 2),                         # solo (2,2) (upper lanes zero-weighted)
]
NCH = len(CHUNKS)  # 14


def replace_range_clears(nc):
    """The For_i back-edge resets loop semaphores with an InstISA
    EVENT_SEMAPHORE_RANGE_CLEAR, which this walrus rejects ('ISA wrong
    length').  Replace each with per-semaphore EventSemaphore writes."""
    import re
    n = 0
    for f in nc.m.functions:
        for bb in f.blocks:
            insts = bb.instructions
            new = []
            changed = False
            for inst in insts:
                if (type(inst).__name__ == "InstISA"
                        and getattr(inst, "isa_opcode", None) == 176):
                    m = re.search(r"range_first=(\d+) range_last=(\d+)",
                                  inst.concise())
                    lo, hi = int(m.group(1)), int(m.group(2))
                    si = inst.sync_info
                    waits = list(si.on_wait) if si is not None else []
                    upds = list(si.on_update) if si is not None else []
                    ids = list(range(lo, hi + 1))
                    for k, sid in enumerate(ids):
                        n += 1
                        ev = mybir.InstEventSemaphore(
                            name=f"rangeclr-{n}", ins=[], outs=[])
                        ev.engine = inst.engine
                        ow = waits if k == 0 else []
                        ou = [mybir.SyncUpdate(
                            sync_type="semaphore", id=sid,
                            update_mode="sem-wr-imm", update_value=0)]
                        if k == len(ids) - 1:
                            ou = ou + upds
                        ev.sync_info = mybir.SyncInfo(on_wait=ow, on_update=ou)
                        new.append(ev)
                    changed = True
                else:
                    new.append(inst)
            if changed:
                insts[:] = new
    return n


def split_multi_waits(nc):
    """This walrus build allows at most ONE sem wait per instruction
    ('Too many sync wait commands').  Hoist extra waits onto same-engine
    nops inserted immediately before the instruction."""
    ctr = 0
    for f in nc.m.functions:
        for bb in f.blocks:
            insts = bb.instructions
            new = []
            changed = False
            for inst in insts:
                si = inst.sync_info
                if si is not None and si.on_wait and len(si.on_wait) > 1:
                    waits = list(si.on_wait)
                    for w in waits[:-1]:
                        ctr += 1
                        nop = mybir.InstNoOp(name=f"splitw-{ctr}", ins=[], outs=[])
                        nop.engine = inst.engine
                        nop.sync_info = mybir.SyncInfo(on_wait=[w], on_update=[])
                        new.append(nop)
                    inst.sync_info = mybir.SyncInfo(
                        on_wait=[waits[-1]], on_update=list(si.on_update))
                    changed = True
                new.append(inst)
            if changed:
                insts[:] = new
    return ctr


class TC(tile.TileContext):
    def __exit__(self, *a):
        r = super().__exit__(*a)
        if a[0] is None:
            replace_range_clears(self.nc)
            split_multi_waits(self.nc)
        return r


def build_nc(R=1):
    nc = bass.Bass("TRN2", target_bir_lowering=False, debug=False)

    p_d = [nc.dram_tensor(f"p{i}", [128, PLANE], BF16, kind="ExternalInput")
           for i in range(3)]
    w1_d = nc.dram_tensor("w1", [128, NCH * 384], BF16, kind="ExternalInput")
    wfvh_d = nc.dram_tensor("wfvh", [128, NCH * 128], BF16, kind="ExternalInput")
    wfvl_d = nc.dram_tensor("wfvl", [64, NCH * 128], BF16, kind="ExternalInput")
    wpth_d = nc.dram_tensor("wpth", [128, 192], BF16, kind="ExternalInput")
    wptl_d = nc.dram_tensor("wptl", [64, 192], BF16, kind="ExternalInput")
    onc_d = nc.dram_tensor("onc", [128, 1], BF16, kind="ExternalInput")
    onr_d = nc.dram_tensor("onr", [1, 129], BF16, kind="ExternalInput")
    temp_d = nc.dram_tensor("temp", [1, 192], F32, kind="ExternalInput")
    az_d = nc.dram_tensor("azero", [128, 192], BF16, kind="ExternalInput")
    y_d = nc.dram_tensor("y", [DIM, HW], F32, kind="ExternalOutput")

    with TC(nc) as tc:
        import contextlib
        stk = contextlib.ExitStack()
        with stk:
            small = stk.enter_context(tc.tile_pool(name="small", bufs=1))
            wpool = stk.enter_context(tc.tile_pool(name="wpool", bufs=1))

            w1 = wpool.tile([128, NCH * 384], BF16, name="w1")
            wfvh = wpool.tile([128, NCH * 128], BF16, name="wfvh")
            wfvl = wpool.tile([64, NCH * 128], BF16, name="wfvl")
            w2 = wpool.tile([128, NCH * 192], BF16, name="w2")
            wpth = small.tile([128, 192], BF16, name="wpth")
            wptl = small.tile([64, 192], BF16, name="wptl")
            onc = small.tile([128, 1], BF16, name="onc")
            onr = small.tile([1, 129], BF16, name="onr")
            temp = small.tile([1, 192], F32, name="temp")

            def body(it):
                nc.sync.dma_start(w1[:], w1_d.ap())
                nc.sync.dma_start(wfvh[:], wfvh_d.ap())
                nc.sync.dma_start(wfvl[:], wfvl_d.ap())
                nc.sync.dma_start(wpth[:], wpth_d.ap())
                nc.sync.dma_start(wptl[:], wptl_d.ap())
                nc.sync.dma_start(onc[:], onc_d.ap())
                nc.sync.dma_start(onr[:], onr_d.ap())
                nc.sync.dma_start(temp[:], temp_d.ap())

                with contextlib.ExitStack() as conv_stk:
                    win = conv_stk.enter_context(tc.tile_pool(name="win", bufs=4))
                    qkp = conv_stk.enter_context(tc.tile_pool(name="qkp", bufs=3))
                    sqp_pool = conv_stk.enter_context(tc.tile_pool(name="sqp", bufs=3))
                    att = conv_stk.enter_context(tc.tile_pool(name="att", bufs=1))
                    gps_stk = conv_stk.enter_context(contextlib.ExitStack())
                    gps = gps_stk.enter_context(
                        tc.tile_pool(name="gps", bufs=1, space="PSUM"))
                    mm_stk = contextlib.ExitStack()
                    cps = mm_stk.enter_context(
                        tc.tile_pool(name="cps", bufs=2, space="PSUM"))

                    g_hi = gps.tile([128, 192], F32, name="g_hi")
                    g_lo = gps.tile([64, 192], F32, name="g_lo")
                    ssq_ps = gps.tile([1, 384], F32, name="ssq_ps")

                    pend_g = []
                    pend_ssq = []

                    def emit_g(qk_sb, y):
                        nc.tensor.matmul(g_hi[:], qk_sb[:, 0:128],
                                         qk_sb[:, 192:384],
                                         start=(y == 0), stop=(y == 127))
                        nc.tensor.matmul(g_lo[:], qk_sb[:, 128:192],
                                         qk_sb[:, 192:384],
                                         start=(y == 0), stop=(y == 127))

                    def emit_ssq(sq_sb, y):
                        nc.tensor.matmul(ssq_ps[:], onc[:], sq_sb[:],
                                         start=(y == 0), stop=(y == 127))

                    # ---- pass 1: qk conv rows + Gram/ssq accumulation ----
                    for g in range(32):
                        wt = []
                        for i in range(3):
                            t = win.tile([128, 780], BF16, name=f"wn{i}",
                                         tag=f"wn{i}")
                            nc.sync.dma_start(
                                t[:], p_d[i].ap()[:, 4 * g * PW:(4 * g + 6) * PW])
                            wt.append(t)
                        for r in range(4):
                            y = 4 * g + r
                            qk_ps = cps.tile([128, 384], F32, name="qk_ps",
                                             tag="qk_ps")
                            for c, (pl, dy, dx) in enumerate(CHUNKS):
                                o = (r + dy) * PW + dx
                                nc.tensor.matmul(
                                    qk_ps[:], wt[pl][:, o:o + 128],
                                    w1[:, c * 384:(c + 1) * 384],
                                    start=(c == 0), stop=(c == NCH - 1))
                            if pend_g:
                                emit_g(*pend_g.pop(0))
                            if len(pend_ssq) > 1:
                                emit_ssq(*pend_ssq.pop(0))
                            qk_sb = qkp.tile([128, 384], BF16, name="qk_sb",
                                             tag="qk_sb")
                            nc.scalar.copy(qk_sb[:], qk_ps[:])
                            sq_sb = sqp_pool.tile([128, 384], BF16, name="sq_sb",
                                                  tag="sq_sb")
                            nc.scalar.square(sq_sb[:], qk_sb[:])
                            pend_g.append((qk_sb, y))
                            pend_ssq.append((sq_sb, y))

                    while pend_g:
                        emit_g(*pend_g.pop(0))
                    while pend_ssq:
                        emit_ssq(*pend_ssq.pop(0))
                    mm_stk.close()  # release conv matmul PSUM banks

                    # ---- attention finalize ----
                    aps_stk = gps_stk.enter_context(contextlib.ExitStack())
                    aps = aps_stk.enter_context(
                        tc.tile_pool(name="aps", bufs=1, space="PSUM"))

                    ssq_sb = att.tile([1, 384], F32, name="ssq_sb")
                    nc.vector.tensor_copy(ssq_sb[:], ssq_ps[:])
                    norm = att.tile([1, 384], F32, name="norm")
                    nc.scalar.sqrt(norm[:], ssq_sb[:])
                    nc.vector.tensor_scalar_max(norm[:], norm[:], 1e-12)
                    rn = att.tile([1, 384], F32, name="rn")
                    nc.vector.reciprocal(rn[:], norm[:])
                    sk_r = att.tile([1, 192], BF16, name="sk_r")
                    nc.vector.tensor_copy(sk_r[:], rn[:, 192:384])
                    sq_f = att.tile([1, 192], F32, name="sq_f")
                    nc.vector.tensor_mul(sq_f[:], rn[:, 0:192], temp[:])

                    bck_ps = aps.tile([128, 192], F32, name="bck_ps")
                    nc.tensor.matmul(bck_ps[:], onr[0:1, 0:128], sk_r[:],
                                     start=True, stop=True)
                    bck_sb = att.tile([128, 192], F32, name="bck_sb")
                    nc.scalar.copy(bck_sb[:], bck_ps[:])

                    sqp_hi = att.tile([128, 1], F32, name="sqp_hi")
                    sqp_lo = att.tile([64, 1], F32, name="sqp_lo")
                    nc.scalar.dma_start(sqp_hi[:, 0:1], sq_f[0:1, 0:128])
                    nc.scalar.dma_start(sqp_lo[:, 0:1], sq_f[0:1, 128:192])

                    gsc_hi = att.tile([128, 192], F32, name="gsc_hi")
                    nc.vector.tensor_mul(gsc_hi[:], g_hi[:], bck_sb[:])
                    gsc_lo = att.tile([64, 192], F32, name="gsc_lo")
                    nc.vector.tensor_mul(gsc_lo[:], g_lo[:], bck_sb[0:64, :])

                    mneg_hi = att.tile([128, 1], F32, name="mneg_hi")
                    mneg_lo = att.tile([64, 1], F32, name="mneg_lo")
                    bias_hi = att.tile([128, 1], F32, name="bias_hi")
                    bias_lo = att.tile([64, 1], F32, name="bias_lo")
                    den_hi = att.tile([128, 1], F32, name="den_hi")
                    den_lo = att.tile([64, 1], F32, name="den_lo")
                    e_hi = att.tile([128, 32], F32, name="e_hi")
                    e_lo = att.tile([64, 32], F32, name="e_lo")
                    a_hi = att.tile([128, 192], BF16, name="a_hi")
                    a_lo = att.tile([64, 192], BF16, name="a_lo")
                    nc.sync.dma_start(a_hi[:], az_d.ap())
                    nc.sync.dma_start(a_lo[:], az_d.ap()[0:64, :])

                    for h in range(HEADS):
                        if h < 4:
                            rows = slice(h * 32, (h + 1) * 32)
                            gsc, mneg, bias, den, e, a, sqv = (
                                gsc_hi, mneg_hi, bias_hi, den_hi, e_hi, a_hi,
                                sqp_hi)
                        else:
                            rows = slice((h - 4) * 32, (h - 3) * 32)
                            gsc, mneg, bias, den, e, a, sqv = (
                                gsc_lo, mneg_lo, bias_lo, den_lo, e_lo, a_lo,
                                sqp_lo)
                        gs = gsc[rows, h * 32:(h + 1) * 32]
                        nc.vector.tensor_reduce(mneg[rows, :], gs, axis=AX.X,
                                                op=mybir.AluOpType.max,
                                                negate=True)
                        nc.vector.tensor_mul(bias[rows, :], mneg[rows, :],
                                             sqv[rows, :])
                        nc.scalar.activation(e[rows, :], gs, AF.Exp,
                                             bias=bias[rows, :],
                                             scale=sqv[rows, :],
                                             accum_out=den[rows, :])
                        nc.vector.reciprocal(den[rows, :], den[rows, :])
                        nc.vector.tensor_scalar_mul(
                            a[rows, h * 32:(h + 1) * 32], e[rows, :],
                            den[rows, :])

                    ct_ps_hi = aps.tile([128, 192], F32, name="ct_ps_hi")
                    ct_ps_lo = aps.tile([64, 192], F32, name="ct_ps_lo")
                    nc.tensor.matmul(ct_ps_hi[:], a_hi[:, 0:128], wpth[:],
                                     start=True, stop=False)
                    nc.tensor.matmul(ct_ps_hi[:], a_lo[:, 0:128], wptl[:],
                                     start=False, stop=True)
                    nc.tensor.matmul(ct_ps_lo[:], a_hi[:, 128:192], wpth[:],
                                     start=True, stop=False)
                    nc.tensor.matmul(ct_ps_lo[:], a_lo[:, 128:192], wptl[:],
                                     start=False, stop=True)
                    ct_sb_hi = att.tile([128, 192], BF16, name="ct_sb_hi")
                    ct_sb_lo = att.tile([64, 192], BF16, name="ct_sb_lo")
                    nc.vector.tensor_copy(ct_sb_hi[:], ct_ps_hi[:])
                    nc.vector.tensor_copy(ct_sb_lo[:], ct_ps_lo[:])

                    gps_stk.close()  # release gram PSUM banks (keeps aps)

                    # ---- fold attention into v-conv weights: Wy^T chunks ----
                    with contextlib.ExitStack() as wy_stk:
                        wyp = wy_stk.enter_context(
                            tc.tile_pool(name="wyp", bufs=2, space="PSUM"))
                        for c in range(NCH):
                            wy_ps = wyp.tile([128, 192], F32, name="wy_ps",
                                             tag="wy_ps")
                            nc.tensor.matmul(wy_ps[:],
                                             wfvh[:, c * 128:(c + 1) * 128],
                                             ct_sb_hi[:],
                                             start=True, stop=False)
                            nc.tensor.matmul(wy_ps[:],
                                             wfvl[:, c * 128:(c + 1) * 128],
                                             ct_sb_lo[:],
                                             start=False, stop=True)
                            nc.scalar.copy(w2[:, c * 192:(c + 1) * 192],
                                           wy_ps[:])
                    aps_stk.close()

                    # ---- pass 2: y = conv3x3(x, Wy), streamed out ----
                    fin = conv_stk.enter_context(tc.tile_pool(name="fin", bufs=3))
                    yps = conv_stk.enter_context(
                        tc.tile_pool(name="yps", bufs=2, space="PSUM"))
                    for g in range(32):
                        wt = []
                        for i in range(3):
                            t = win.tile([128, 780], BF16, name=f"wn{i}",
                                         tag=f"wn{i}")
                            nc.sync.dma_start(
                                t[:], p_d[i].ap()[:, 4 * g * PW:(4 * g + 6) * PW])
                            wt.append(t)
                        wv = [t[:].rearrange("p (r c) -> p r c", r=6)
                              for t in wt]
                        y_ps_hi = yps.tile([128, 512], F32, name="y_ps_hi",
                                           tag="y_ps_hi")
                        y_ps_lo = yps.tile([64, 512], F32, name="y_ps_lo",
                                           tag="y_ps_lo")
                        for c, (pl, dy, dx) in enumerate(CHUNKS):
                            rhs = wv[pl][:, dy:dy + 4, dx:dx + 128]
                            nc.tensor.matmul(y_ps_hi[:],
                                             w2[:, c * 192:c * 192 + 128],
                                             rhs, start=(c == 0),
                                             stop=(c == NCH - 1))
                        for c, (pl, dy, dx) in enumerate(CHUNKS):
                            rhs = wv[pl][:, dy:dy + 4, dx:dx + 128]
                            nc.tensor.matmul(y_ps_lo[:],
                                             w2[:, c * 192 + 128:c * 192 + 192],
                                             rhs, start=(c == 0),
                                             stop=(c == NCH - 1))
                        y_sb_hi = fin.tile([128, 512], F32, name="y_sb_hi",
                                           tag="y_sb_hi")
                        y_sb_lo = fin.tile([64, 512], F32, name="y_sb_lo",
                                           tag="y_sb_lo")
                        nc.scalar.copy(y_sb_hi[:], y_ps_hi[:])
                        nc.vector.tensor_copy(y_sb_lo[:], y_ps_lo[:])
                        cs = slice(512 * g, 512 * (g + 1))
                        nc.scalar.dma_start(y_d.ap()[0:128, cs], y_sb_hi[:])
                        nc.scalar.dma_start(y_d.ap()[128:192, cs], y_sb_lo[:])

            if R == 1:
                body(0)
            else:
                with tc.For_i(0, R, 1) as it:
                    body(it)

    # NEFF-cache buster (see baseline): distinct HLO per distinct BIR.
    import zlib
    h = zlib.crc32(nc.to_json_bytes()) % 997 + 1
    nc.dram_tensor("cachebust", [1, h], F32, kind="ExternalInput")
    return nc


class PjrtRunner:
    """Build the jitted SPMD executable once; allow repeated timed runs."""

    def __init__(self, nc, n_cores=8):
        from jax.sharding import Mesh, PartitionSpec
        from jax.experimental.shard_map import shard_map
        bass2jax.install_neuronx_cc_hook()
        self.nc = nc
        self.n_cores = n_cores
        partition_name = (nc.partition_id_tensor.name
                          if nc.partition_id_tensor else None)
        in_names, out_names, out_avals = [], [], []
        for alloc in nc.m.functions[0].allocations:
            if not isinstance(alloc, mybir.MemoryLocationSet):
                continue
            name = alloc.memorylocations[0].name
            if alloc.kind == "ExternalInput":
                if name != partition_name:
                    in_names.append(name)
            elif alloc.kind == "ExternalOutput":
                out_names.append(name)
                out_avals.append(jax.core.ShapedArray(
                    tuple(alloc.tensor_shape), mybir.dt.np(alloc.dtype)))
        self.in_names, self.out_names, self.out_avals = (
            in_names, out_names, out_avals)
        n_params = len(in_names)
        all_in_names = list(in_names) + list(out_names)
        if partition_name is not None:
            all_in_names.append(partition_name)

        def _body(*args):
            operands = list(args)
            if partition_name is not None:
                operands.append(bass2jax.partition_id_tensor())
            outs = bass2jax._bass_exec_p.bind(
                *operands,
                out_avals=tuple(out_avals),
                in_names=tuple(all_in_names),
                out_names=tuple(out_names),
                lowering_input_output_aliases=(),
                sim_require_finite=False,
                sim_require_nnan=False,
                nc=nc,
            )
            return tuple(outs)

        devices = jax.devices()[:n_cores]
        self.mesh = Mesh(np.asarray(devices), ("core",))
        in_specs = (PartitionSpec("core"),) * (n_params + len(out_names))
        out_specs = (PartitionSpec("core"),) * len(out_names)
        self.sharded = jax.jit(shard_map(
            _body, mesh=self.mesh, in_specs=in_specs, out_specs=out_specs,
            check_rep=False))

    def prepare(self, in_maps):
        n_cores = self.n_cores
        shapes = {}
        for alloc in self.nc.m.functions[0].allocations:
            if (isinstance(alloc, mybir.MemoryLocationSet)
                    and alloc.kind == "ExternalInput"):
                shapes[alloc.memorylocations[0].name] = (
                    tuple(alloc.tensor_shape), mybir.dt.np(alloc.dtype))
        def get(m, name):
            if name in m:
                return np.ascontiguousarray(np.asarray(m[name]))
            shp, dt = shapes[name]
            return np.zeros(shp, dt)
        per_core = [[get(m, name) for name in self.in_names] for m in in_maps]
        concat_in = [np.concatenate([per_core[c][i] for c in range(n_cores)],
                                    axis=0)
                     for i in range(len(self.in_names))]
        concat_zeros = [np.zeros((n_cores * a.shape[0], *a.shape[1:]), a.dtype)
                        for a in self.out_avals]
        self.dev_in = [jax.device_put(a) for a in concat_in]
        self.dev_zeros = [jax.device_put(a) for a in concat_zeros]

    def run(self):
        outs = self.sharded(*self.dev_in, *self.dev_zeros)
        jax.block_until_ready(outs)
        return outs

    def results(self, outs):
        n_cores = self.n_cores
        return [
            {name: np.asarray(outs[i]).reshape(
                n_cores, *self.out_avals[i].shape)[c]
             for i, name in enumerate(self.out_names)}
            for c in range(n_cores)
        ]


_RUNNERS = {}


def _get_runner(R=1):
    if R not in _RUNNERS:
        _RUNNERS[R] = PjrtRunner(build_nc(R), B)
    return _RUNNERS[R]


def _host_prep(x, W_qkv, W_dw, W_proj, temperature):
    import ml_dtypes
    BD = ml_dtypes.bfloat16
    x = np.asarray(x, np.float32)
    W_qkv = np.asarray(W_qkv, np.float64)
    W_dw = np.asarray(W_dw, np.float64)
    W_proj = np.asarray(W_proj, np.float32)
    temperature = np.asarray(temperature, np.float32)

    # fused conv weights: W_f[o,dy,dx,i] = sum_m W_dw[o,m,dy,dx] W_qkv[m,i]
    wd = W_dw.transpose(0, 2, 3, 1).reshape(576 * 9, 576)
    wf4 = (wd @ W_qkv[:, :, 0, 0]).reshape(576, 3, 3, DIM).astype(np.float32)

    # lane table per chunk: list of (ci, dy, dx) for lanes 0..127 (None = zero)
    def lanes(c):
        pl, dy0, dx0 = CHUNKS[c]
        if pl == 0:
            return [(i, dy0, dx0) for i in range(128)]
        if c in (9, 10, 11):
            return ([(128 + j, dy0, 0) for j in range(64)]
                    + [(128 + j, dy0, 1) for j in range(64)])
        if c == 12:
            return ([(128 + j, 0, 2) for j in range(64)]
                    + [(128 + j, 1, 2) for j in range(64)])
        return [(128 + j, 2, 2) for j in range(64)] + [None] * 64

    w1 = np.zeros((128, NCH, 384), np.float32)
    wfv = np.zeros((192, NCH, 128), np.float32)
    for c in range(NCH):
        for l, ln in enumerate(lanes(c)):
            if ln is None:
                continue
            ci, dy, dx = ln
            w1[l, c, :] = wf4[0:384, dy, dx, ci]
            wfv[:, c, l] = wf4[384:576, dy, dx, ci]
    w1 = w1.reshape(128, NCH * 384).astype(BD)
    wfvh = wfv[0:128].reshape(128, NCH * 128).astype(BD)
    wfvl = wfv[128:192].reshape(64, NCH * 128).astype(BD)

    wpt = W_proj[:, :, 0, 0].T.astype(np.float32)  # [c_in, o]
    wpth = wpt[0:128].astype(BD)
    wptl = np.ascontiguousarray(wpt[128:192]).astype(BD)
    temp_ext = np.repeat(temperature.reshape(HEADS), 32).astype(
        np.float32).reshape(1, 192)

    common = {
        "w1": w1, "wfvh": wfvh, "wfvl": wfvl,
        "wpth": wpth, "wptl": wptl,
        "onc": np.ones((128, 1), BD), "onr": np.ones((1, 129), BD),
        "temp": temp_ext, "azero": np.zeros((128, 192), BD),
    }

    in_maps = []
    for b in range(B):
        xp = np.zeros((DIM, PW, PW), np.float32)
        xp[:, 1:129, 1:129] = x[b]
        lo = xp[128:192]
        l2 = np.zeros((128, PW, PW), np.float32)
        l2[0:64] = lo
        l2[64:128, :, 0:PW - 1] = lo[:, :, 1:PW]
        l3 = np.zeros((128, PW, PW), np.float32)
        l3[0:64] = lo
        l3[64:128, 0:PW - 1, :] = lo[:, 1:PW, :]
        m = dict(common)
        m["p0"] = xp[0:128].reshape(128, PLANE).astype(BD)
        m["p1"] = l2.reshape(128, PLANE).astype(BD)
        m["p2"] = l3.reshape(128, PLANE).astype(BD)
        in_maps.append(m)
    return in_maps


def kernel(x, W_qkv, W_dw, W_proj, temperature):
    in_maps = _host_prep(x, W_qkv, W_dw, W_proj, temperature)
    r = _get_runner(1)
    r.prepare(in_maps)
    res = r.results(r.run())
    out = np.stack([res[b]["y"].reshape(DIM, 128, 128) for b in range(B)])
    return out.astype(np.float32)


def measure_hw_time_ns(inputs, R=17, n_pairs=10):
    """Paired interleaved timing of R=1 vs R=R NEFFs; returns est ns/iter."""
    in_maps = _host_prep(**inputs)
    r1 = _get_runner(1)
    rR = _get_runner(R)
    r1.prepare(in_maps)
    rR.prepare(in_maps)
    r1.run(); rR.run()
    d1, dR = [], []
    for _ in range(n_pairs):
        t0 = time.perf_counter(); r1.run(); d1.append(time.perf_counter() - t0)
        t0 = time.perf_counter(); rR.run(); dR.append(time.perf_counter() - t0)
    d1 = np.array(d1); dR = np.array(dR)
    est_med = (np.median(dR) - np.median(d1)) / (R - 1) * 1e9
    est_min = (dR.min() - d1.min()) / (R - 1) * 1e9
    return est_med, est_min
